# revision 25
# baseline (speedup 1.0000x reference)
"""Otsu-threshold binary region proposal kernel for Trainium2 (8 NeuronCores).

Algorithm (per image of 224*224 pixels, 512 images total, data-parallel over
8 cores / 64 images per core):

  reference:  cam = floor(x*255); per-image 256-bin histogram; Otsu threshold
              via argmax of inter-class variance restricted to [vmin, vmax);
              roi = (cam > th), 0 for degenerate images.

Device pass A (histogram + cam cache):
  Thermometer decomposition: with hi = cam >> 4, lo = cam & 15,
      R[tau, sigma] = sum_p colA_tau(p) * colB_sigma(p)
  colA_tau ~ [cam >= 16*tau] (pack-major, stationary), colB_sigma ~ [lo >=
  sigma] (cut-major, moving); 16x16 pair counts accumulate on the PE as one
  [128,128]x[128,128] matmul per 8-chunk pack (block-diagonal; host sums the
  c'==c'' blocks).  Engine tuning (all probe-measured on HW):
   - DVE feature cols run 4x (both contiguous AND [1,8]-run strided) ONLY
     when the strided pack count is EVEN — hence the +1 pack pad.
   - DVE does prep (lone 255* mult — op0/op1 chains are FUSED with no
     intermediate rounding, which breaks floor(fl(255x)) — then the 2^23
     magic add-add + the lo AND) plus ~7.5 A rows and all 15 B rows.
   - ACT (1x always, ~1270ns/col) takes the other ~7.5 A rows as Sign
     (+-1 coded, decoded via marginals) alternating the split by group
     parity to balance both engines, plus the ci8 u8 convert and the
     deferred psum->SBUF copies (deferred one full group so ACT's in-order
     queue never head-of-line blocks feature rows on a matmul wait).
   - GPSIMD tensor ops are ~15.5ns/elem (useless); it only memsets the
     constant ones-rows ONCE per physical buffer in the prologue, keeping
     its slow variable memsets off the steady-state critical path.
  Pass A also emits cam as uint8 (ci8) to DRAM so pass B never re-reads x.

Host (exact float32, mirrors jax reference op-for-op):
  decode W -> 256-bin histogram -> cumsums -> inter-class variance -> argmax
  -> th.  mask = (cam > th) == (ci8 >= th+1), exact in integers: no float
  cut table needed.  Degenerate images get th+1 = 256 (never fires).

Device pass B (mask): mask = (ci8 >= thp1) as uint8; reads 3.2MB instead of
the 12.8MB of x, writes 3.2MB.

floor() trick (no floor ALU op): t = fl(255x + (2^23 - 0.5)); ci = t - 2^23
= floor(fl(255x)) exactly (fp32 ulp 1.0 at 2^23 forces round-to-integer;
the -0.5 biases the tie so exact integers round down correctly... for
s = fl(255x) integer the only case is s=0 -> t = 2^23 - 0.5 rounds to 2^23
(ties-to-even) -> ci = 0 correct).
"""

import math
import os
import sys

import numpy as np

sys.path.insert(0, "/opt/trn_rl_repo")

import concourse.bacc as bacc
import concourse.bass as bass  # noqa: F401
import concourse.mybir as mybir
from concourse.bass_utils import run_bass_kernel_spmd
from concourse.tile import TileContext

# ---------------------------------------------------------------------------
# Problem geometry (hardcoded per spec)
B, N, H, W_IMG = 64, 8, 224, 224
PIX = H * W_IMG              # 50176
PARTS = 128
CPI = PIX // PARTS           # 392 chunks (columns) per image
N_CORES = 8
IMGS_PER_CORE = (B // N_CORES) * N      # 64
NBINS = 256

# Tunables -----------------------------------------------------------------
GROUP = 3          # images per thermo group
# Probe-measured (ns, FD1176-equivalent): DVE is_ge bf16-out ~457 at 4x
# (contiguous OR [1,8]-run strided) but degrades to ~755 (2x) when ANOTHER
# engine is concurrently writing the same SBUF tile (port contention — same
# mechanism the GPSIMD note below describes).  DVE u8/fp32 ops ~755 (2x_2p);
# ACT ~1260 per col (1x always); GPSIMD tensor_scalar ~15.5ns/elem (NEVER
# use); GPSIMD memset ~free.  So: feature columns go DVE-first, ACT takes
# the A-row overflow, GPSIMD only memsets, and emission order staggers the
# writers of each tile (GP memsets, then ACT's Sign rows, then DVE's rows
# after filler work) to keep single-writer-per-tile windows.
# A-plane (pack-major, stationary): rows 1..A_DVE_ROWS on DVE, rows
# A_DVE_ROWS+1..15 on ACT (Sign, +-1 coded).  B-plane (cut-major, moving):
# all rows 1..15 on DVE with the lo-AND folded into each op (and,is_ge
# chain); ci8 conversion on ACT.
A_DVE_ROWS = 7
B_ACT_ROWS = 0
MGROUP = 16        # images per pass-B tile group

FP32 = mybir.dt.float32
BF16 = mybir.dt.bfloat16
I16 = mybir.dt.int16
U8 = mybir.dt.uint8
ALU = mybir.AluOpType
ACTF = mybir.ActivationFunctionType
MAGIC = 8388608.0  # 2^23


def _enc_pm(nimg=IMGS_PER_CORE):
    """Which rows are +-1 coded (ACT Sign), per image (the DVE/ACT A-row
    split alternates by group parity)."""
    g = np.arange(nimg) // GROUP
    a_dve = A_DVE_ROWS + (g & 1)                       # [img]
    rows = np.arange(16)[None, :]
    encA = rows > a_dve[:, None]                       # [img, 16]
    encB = np.zeros((nimg, 16), dtype=bool)
    if B_ACT_ROWS:
        encB[:, 16 - B_ACT_ROWS:] = True
    return encA, encB


# ---------------------------------------------------------------------------
# Pass A: histogram + ci8 cache
def build_hist_nc(nimg=IMGS_PER_CORE, cpi=CPI, group=GROUP):
    assert cpi % 8 == 0
    nc = bacc.Bacc("TRN2", target_bir_lowering=False, debug=False)
    x_d = nc.dram_tensor("x", [PARTS, nimg, cpi], FP32, kind="ExternalInput")
    w_d = nc.dram_tensor("w_raw", [nimg, PARTS, PARTS], FP32, kind="ExternalOutput")
    c8_d = nc.dram_tensor("ci8", [PARTS, nimg, cpi], I16, kind="ExternalOutput")

    with TileContext(nc) as tc:
        with (
            tc.tile_pool(name="const", bufs=1) as cpool,
            tc.tile_pool(name="xin", bufs=3) as xpool,
            tc.tile_pool(name="prep", bufs=3) as ppool,
            tc.tile_pool(name="thermo", bufs=2) as tpool,
            tc.tile_pool(name="psum", bufs=6, space="PSUM") as qpool,
        ):
            # per-partition biases for ACT Sign rows: sign(v - cut + 0.5)
            nACT = 15 - A_DVE_ROWS
            act_bias = cpool.tile([PARTS, max(nACT + 1, 1)], FP32, tag="abias")
            ab_idx = {}
            j = 0
            for tau in range(A_DVE_ROWS, 16):
                nc.vector.memset(act_bias[:, j:j + 1], 0.5 - 16 * tau)
                ab_idx[("A", tau)] = j
                j += 1

            n_groups = math.ceil(nimg / group)

            def emit_load(g):
                g0 = g * group
                g1 = min(g0 + group, nimg)
                gw = (g1 - g0) * cpi
                x_t = xpool.tile([PARTS, group * cpi], FP32, tag="x")
                nc.sync.dma_start(
                    out=x_t[:, :gw],
                    in_=bass.AP(x_d, g0 * cpi, [[nimg * cpi, PARTS], [1, gw]]),
                )
                return x_t

            def emit_prep(g, x_t):
                """DVE floor prep.  CAUTION: DVE op0/op1 chains are FUSED
                (no intermediate fp32 rounding — probe-verified), so the
                255* multiply must be a LONE op to round fl(255x) before
                the magic add; the add-add chain itself is safe fused
                (s-0.5 exact, convert rounds nearest) or unfused (classic
                2^23 magic)."""
                g0 = g * group
                g1 = min(g0 + group, nimg)
                gw = (g1 - g0) * cpi
                # +8 pad: A-plane pack count must be EVEN for DVE 4x mode
                # (odd-count [1,8]-run writes drop to 2x — probe-measured)
                ci = ppool.tile([PARTS, group * cpi + 8], I16, tag="ci")
                nc.vector.tensor_scalar(
                    out=x_t[:, :gw], in0=x_t[:, :gw],
                    scalar1=255.0, scalar2=None, op0=ALU.mult,
                )
                nc.vector.tensor_scalar(
                    out=ci[:, :gw], in0=x_t[:, :gw],
                    scalar1=MAGIC - 0.5, scalar2=-MAGIC,
                    op0=ALU.add, op1=ALU.add,
                )
                lo = ppool.tile([PARTS, group * cpi + 8], I16, tag="lo")
                nc.vector.tensor_scalar(
                    out=lo[:, :gw], in0=ci[:, :gw],
                    scalar1=15, scalar2=None, op0=ALU.bitwise_and,
                )
                return ci, lo

            x_cur = emit_load(0)
            x_nxt = emit_load(1) if n_groups > 1 else None
            # ones rows are CONSTANT: write them once per physical tile
            # buffer (pool rotation: 2 requests = both buffers), then never
            # again — keeps the slow, variable GPSIMD memsets off the
            # steady-state critical path (they were serializing each
            # group's A/B writers via whole-tile WAW tracking).
            for _ in range(2):
                A_pre = tpool.tile([PARTS, group * cpi // 8 + 1, 16, 8], BF16, tag="A")
                B_pre = tpool.tile([PARTS, 16, group * cpi], BF16, tag="B")
                nc.gpsimd.memset(A_pre[:, :, 0, :], 1.0)
                nc.gpsimd.memset(B_pre[:, 0, :], 1.0)

            cur = emit_prep(0, x_cur)
            pend = []
            for g in range(n_groups):
                g0 = g * group
                g1 = min(g0 + group, nimg)
                gw = (g1 - g0) * cpi
                gw8 = gw // 8
                gw8p = gw8 + (gw8 & 1)   # even pack count for DVE 4x
                ci, lo = cur

                if g + 2 < n_groups:
                    x_nxt2 = emit_load(g + 2)
                else:
                    x_nxt2 = None

                A_t = tpool.tile([PARTS, group * cpi // 8 + 1, 16, 8], BF16, tag="A")
                B_t = tpool.tile([PARTS, 16, group * cpi], BF16, tag="B")
                # alternate the DVE/ACT A-row split by group parity to
                # balance the two engines (7.5 rows each on average)
                a_dve = A_DVE_ROWS + (g & 1)

                ci_v = ci[:, :gw8p * 8].rearrange("p (a b) -> p a b", b=8)
                # ACT: Sign rows of the A plane + the ci8 u8 conversion.
                # Emitted before DVE's blocks so ACT is done with the A tile
                # by the time DVE's A rows issue (single-writer windows).
                for tau in range(a_dve + 1, 16):
                    j = ab_idx[("A", tau)]
                    nc.scalar.activation(
                        out=A_t[:, :gw8p, tau, :], in_=ci_v,
                        func=ACTF.Sign,
                        bias=act_bias[:, j:j + 1],
                        scale=1.0,
                    )
                nc.sync.dma_start(
                    out=bass.AP(c8_d, g0 * cpi, [[nimg * cpi, PARTS], [1, gw]]),
                    in_=ci[:, :gw],
                )

                # DVE: B plane rows
                for sg in range(1, 16):
                    nc.vector.tensor_scalar(
                        out=B_t[:, sg, :gw], in0=lo[:, :gw],
                        scalar1=sg, scalar2=None, op0=ALU.is_ge,
                    )

                # filler between DVE's B and A blocks: next group's prep
                if g + 1 < n_groups:
                    cur = emit_prep(g + 1, x_nxt)
                x_nxt = x_nxt2

                # DVE: A plane rows (ACT has finished its A rows by now)
                for tau in range(1, a_dve + 1):
                    nc.vector.tensor_scalar(
                        out=A_t[:, :gw8p, tau, :], in0=ci_v,
                        scalar1=16 * tau, scalar2=None, op0=ALU.is_ge,
                    )

                # PE: per image, 49 packed [128,128] matmuls accumulate in
                # PSUM.  The psum->SBUF copies are DEFERRED one group (psum
                # bufs=8 holds 2 groups) so ACT's in-order queue never
                # head-of-line blocks its next feature rows on an MM wait.
                packs_per_img = cpi // 8
                for i in range(g0, g1):
                    il = i - g0
                    psum_t = qpool.tile([PARTS, PARTS], FP32, tag="ps")
                    for k in range(packs_per_img):
                        p = il * packs_per_img + k
                        nc.tensor.matmul(
                            psum_t[:],
                            A_t[:, p, :, :].rearrange("p a b -> p (a b)"),
                            B_t[:, :, 8 * p:8 * p + 8],
                            start=(k == 0),
                            stop=(k == packs_per_img - 1),
                        )
                    pend.append((i, psum_t))
                while len(pend) > group:
                    i, psum_t = pend.pop(0)
                    w_sb = ppool.tile([PARTS, PARTS], FP32, tag="wsb")
                    nc.scalar.copy(w_sb[:], psum_t[:])
                    nc.sync.dma_start(out=w_d.ap()[i], in_=w_sb[:])
            for i, psum_t in pend:
                w_sb = ppool.tile([PARTS, PARTS], FP32, tag="wsb")
                nc.scalar.copy(w_sb[:], psum_t[:])
                nc.sync.dma_start(out=w_d.ap()[i], in_=w_sb[:])
    nc.finalize()
    return nc


# ---------------------------------------------------------------------------
# Pass B: mask from cached ci8
def build_mask_nc(nimg=IMGS_PER_CORE, cpi=CPI, mgroup=MGROUP):
    nc = bacc.Bacc("TRN2", target_bir_lowering=False, debug=False)
    c8_d = nc.dram_tensor("ci8", [PARTS, nimg, cpi], I16, kind="ExternalInput")
    t_d = nc.dram_tensor("thp1", [PARTS, nimg], FP32, kind="ExternalInput")
    m_d = nc.dram_tensor("mask", [PARTS, nimg, cpi], U8, kind="ExternalOutput")

    with TileContext(nc) as tc:
        with (
            tc.tile_pool(name="cst", bufs=1) as cpool,
            tc.tile_pool(name="cin", bufs=4) as xpool,
            tc.tile_pool(name="mo", bufs=4) as mpool,
        ):
            th_all = cpool.tile([PARTS, nimg], FP32, tag="t")
            nc.sync.dma_start(out=th_all[:], in_=t_d.ap())
            for g0 in range(0, nimg, mgroup):
                g1 = min(g0 + mgroup, nimg)
                gl = g1 - g0
                c_t = xpool.tile([PARTS, mgroup * cpi], I16, tag="c")
                m_t = mpool.tile([PARTS, mgroup * cpi], U8, tag="m")
                nc.sync.dma_start(
                    out=c_t[:, :gl * cpi],
                    in_=bass.AP(
                        c8_d, g0 * cpi,
                        [[nimg * cpi, PARTS], [1, gl * cpi]],
                    ),
                )
                for i in range(g0, g1):
                    il = i - g0
                    nc.vector.tensor_scalar(
                        out=m_t[:, il * cpi:(il + 1) * cpi],
                        in0=c_t[:, il * cpi:(il + 1) * cpi],
                        scalar1=th_all[:, i:i + 1],
                        scalar2=None, op0=ALU.is_ge,
                    )
                nc.sync.dma_start(
                    out=bass.AP(
                        m_d, g0 * cpi,
                        [[nimg * cpi, PARTS], [1, gl * cpi]],
                    ),
                    in_=m_t[:, :gl * cpi],
                )
    nc.finalize()
    return nc


# ---------------------------------------------------------------------------
# Host: decode W, exact-float32 Otsu
def decode_hist(w_raw, nimg=IMGS_PER_CORE, npix=PIX):
    """w_raw [nimg, 128, 128] fp32 -> hist [nimg, 256] int64 (exact).

    Psum row 8*tau+c', col 8*sigma+c'': sum the c'==c'' diagonal blocks."""
    encA, encB = _enc_pm(nimg)               # [img, 16] each
    P128 = np.round(np.asarray(w_raw, np.float64)).astype(np.int64)
    P128 = P128.reshape(nimg, 16, 8, 16, 8)  # [img, tau, c', sigma, c'']
    R = np.einsum("itcsc->its", P128)        # [img, tau, sigma]
    P = npix
    sumB = np.where(encB, (R[:, 0, :] + P) // 2, R[:, 0, :])
    sumA = np.where(encA, (R[:, :, 0] + P) // 2, R[:, :, 0])
    eA = encA[:, :, None]
    eB = encB[:, None, :]
    sA = sumA[:, :, None]
    sB = sumB[:, None, :]
    W = np.where(
        ~eA & ~eB, R,
        np.where(
            eA & ~eB, (R + sB) // 2,
            np.where(~eA & eB, (R + sA) // 2, (R + 2 * sA + 2 * sB - P) // 4),
        ),
    )
    chk = np.where(
        ~eA & ~eB, 0,
        np.where(eA & ~eB, (R + sB) % 2,
                 np.where(~eA & eB, (R + sA) % 2, (R + 2 * sA + 2 * sB - P) % 4)),
    )
    assert not chk.any(), "non-integer decode: device histogram corrupted"
    Wp = np.zeros((nimg, 17, 17), np.int64)
    Wp[:, :16, :16] = W
    hist = (Wp[:, :16, :16] - Wp[:, 1:, :16] - Wp[:, :16, 1:] + Wp[:, 1:, 1:])
    hist = hist.reshape(nimg, 256)
    assert (hist >= 0).all() and (hist.sum(1) == P).all(), "bad histogram"
    return hist


def otsu_f32(hist):
    """Mirror the jax float32 reference exactly. hist [n,256] int64 -> th, bad."""
    f = hist.astype(np.float32)
    centers = np.arange(NBINS, dtype=np.float32)
    w1 = np.cumsum(f, axis=1, dtype=np.float32)
    total = w1[:, -1:]
    s1 = np.cumsum(f * centers, axis=1, dtype=np.float32)
    stot = s1[:, -1:]
    w2 = total - w1
    with np.errstate(divide="ignore", invalid="ignore"):
        m1 = s1 / w1
        m2 = (stot - s1) / w2
        d = m1 - m2
        var12 = (w1 * w2) * (d * d)
    nz = hist > 0
    t = np.arange(NBINS)
    vmin = np.argmax(nz, axis=1)
    vmax = NBINS - 1 - np.argmax(nz[:, ::-1], axis=1)
    valid = (t[None, :] >= vmin[:, None]) & (t[None, :] < vmax[:, None])
    var12 = np.where(valid, var12, np.float32(-1.0))
    th = np.argmax(var12, axis=1)
    th = np.where(th == 0, 1, th)
    th = np.where(th == 255, 254, th)
    bad = vmin == vmax
    return th, bad


# ---------------------------------------------------------------------------
_NC_CACHE = {}


def _get_ncs():
    if "hist" not in _NC_CACHE:
        _NC_CACHE["hist"] = build_hist_nc()
        _NC_CACHE["mask"] = build_mask_nc()
    return _NC_CACHE["hist"], _NC_CACHE["mask"]


def kernel(x: np.ndarray, _profile: dict | None = None) -> np.ndarray:
    x = np.ascontiguousarray(np.asarray(x, dtype=np.float32))
    assert x.shape == (B, N, H, W_IMG)
    nc_hist, nc_mask = _get_ncs()

    bpc = B // N_CORES
    shards = [
        np.ascontiguousarray(
            x[k * bpc:(k + 1) * bpc]
            .reshape(IMGS_PER_CORE, PARTS, CPI)
            .transpose(1, 0, 2)
        )
        for k in range(N_CORES)
    ]
    core_ids = list(range(N_CORES))

    kwargs_a = dict(_profile.get("a", {})) if _profile else {}
    res_a = run_bass_kernel_spmd(
        nc_hist, [{"x": s} for s in shards], core_ids=core_ids, **kwargs_a
    )
    if _profile is not None:
        _profile["res_a"] = res_a

    thp1s = []
    for k in range(N_CORES):
        hist = decode_hist(res_a.results[k]["w_raw"])
        th, bad = otsu_f32(hist)
        thp1 = np.where(bad, np.float32(256.0), (th + 1).astype(np.float32))
        thp1s.append(
            np.ascontiguousarray(
                np.broadcast_to(thp1[None, :], (PARTS, IMGS_PER_CORE))
            ).astype(np.float32)
        )

    kwargs_b = dict(_profile.get("b", {})) if _profile else {}
    res_b = run_bass_kernel_spmd(
        nc_mask,
        [{"ci8": np.asarray(res_a.results[k]["ci8"]), "thp1": thp1s[k]}
         for k in range(N_CORES)],
        core_ids=core_ids,
        **kwargs_b,
    )
    if _profile is not None:
        _profile["res_b"] = res_b

    out = np.empty((B, N, H, W_IMG), np.int32)
    for k in range(N_CORES):
        m = np.asarray(res_b.results[k]["mask"])  # [128, 64, 392] u8
        out[k * bpc:(k + 1) * bpc] = (
            m.astype(np.int32).transpose(1, 0, 2).reshape(bpc, N, H, W_IMG)
        )
    return out


# revision 26
# speedup vs baseline: 1.0042x; 1.0042x over previous
"""Otsu-threshold binary region proposal kernel for Trainium2 (8 NeuronCores).

Algorithm (per image of 224*224 pixels, 512 images total, data-parallel over
8 cores / 64 images per core):

  reference:  cam = floor(x*255); per-image 256-bin histogram; Otsu threshold
              via argmax of inter-class variance restricted to [vmin, vmax);
              roi = (cam > th), 0 for degenerate images.

Device pass A (histogram + cam cache):
  Thermometer decomposition: with hi = cam >> 4, lo = cam & 15,
      R[tau, sigma] = sum_p colA_tau(p) * colB_sigma(p)
  colA_tau ~ [cam >= 16*tau] (pack-major, stationary), colB_sigma ~ [lo >=
  sigma] (cut-major, moving); 16x16 pair counts accumulate on the PE as one
  [128,128]x[128,128] matmul per 8-chunk pack (block-diagonal; host sums the
  c'==c'' blocks).  Engine tuning (all probe-measured on HW):
   - DVE feature cols run 4x (both contiguous AND [1,8]-run strided) ONLY
     when the strided pack count is EVEN — hence the +1 pack pad.
   - DVE does prep (lone 255* mult — op0/op1 chains are FUSED with no
     intermediate rounding, which breaks floor(fl(255x)) — then the 2^23
     magic add-add + the lo AND) plus ~7.5 A rows and all 15 B rows.
   - ACT (1x always, ~1270ns/col) takes the other ~7.5 A rows as Sign
     (+-1 coded, decoded via marginals) alternating the split by group
     parity to balance both engines, plus the ci8 u8 convert and the
     deferred psum->SBUF copies (deferred one full group so ACT's in-order
     queue never head-of-line blocks feature rows on a matmul wait).
   - GPSIMD tensor ops are ~15.5ns/elem (useless); it only memsets the
     constant ones-rows ONCE per physical buffer in the prologue, keeping
     its slow variable memsets off the steady-state critical path.
  Pass A also emits cam as uint8 (ci8) to DRAM so pass B never re-reads x.

Host (exact float32, mirrors jax reference op-for-op):
  decode W -> 256-bin histogram -> cumsums -> inter-class variance -> argmax
  -> th.  mask = (cam > th) == (ci8 >= th+1), exact in integers: no float
  cut table needed.  Degenerate images get th+1 = 256 (never fires).

Device pass B (mask): mask = (ci8 >= thp1) as uint8; reads 3.2MB instead of
the 12.8MB of x, writes 3.2MB.

floor() trick (no floor ALU op): t = fl(255x + (2^23 - 0.5)); ci = t - 2^23
= floor(fl(255x)) exactly (fp32 ulp 1.0 at 2^23 forces round-to-integer;
the -0.5 biases the tie so exact integers round down correctly... for
s = fl(255x) integer the only case is s=0 -> t = 2^23 - 0.5 rounds to 2^23
(ties-to-even) -> ci = 0 correct).
"""

import math
import os
import sys

import numpy as np

sys.path.insert(0, "/opt/trn_rl_repo")

import concourse.bacc as bacc
import concourse.bass as bass  # noqa: F401
import concourse.mybir as mybir
from concourse.bass_utils import run_bass_kernel_spmd
from concourse.tile import TileContext

# ---------------------------------------------------------------------------
# Problem geometry (hardcoded per spec)
B, N, H, W_IMG = 64, 8, 224, 224
PIX = H * W_IMG              # 50176
PARTS = 128
CPI = PIX // PARTS           # 392 chunks (columns) per image
N_CORES = 8
IMGS_PER_CORE = (B // N_CORES) * N      # 64
NBINS = 256

# Tunables -----------------------------------------------------------------
GROUP = 3          # images per thermo group
# Probe-measured (ns, FD1176-equivalent): DVE is_ge bf16-out ~457 at 4x
# (contiguous OR [1,8]-run strided) but degrades to ~755 (2x) when ANOTHER
# engine is concurrently writing the same SBUF tile (port contention — same
# mechanism the GPSIMD note below describes).  DVE u8/fp32 ops ~755 (2x_2p);
# ACT ~1260 per col (1x always); GPSIMD tensor_scalar ~15.5ns/elem (NEVER
# use); GPSIMD memset ~free.  So: feature columns go DVE-first, ACT takes
# the A-row overflow, GPSIMD only memsets, and emission order staggers the
# writers of each tile (GP memsets, then ACT's Sign rows, then DVE's rows
# after filler work) to keep single-writer-per-tile windows.
# A-plane (pack-major, stationary): rows 1..A_DVE_ROWS on DVE, rows
# A_DVE_ROWS+1..15 on ACT (Sign, +-1 coded).  B-plane (cut-major, moving):
# all rows 1..15 on DVE with the lo-AND folded into each op (and,is_ge
# chain); ci8 conversion on ACT.
A_DVE_ROWS = 7
B_ACT_ROWS = 0
MGROUP = 8         # images per pass-B tile group

FP32 = mybir.dt.float32
BF16 = mybir.dt.bfloat16
I16 = mybir.dt.int16
U8 = mybir.dt.uint8
ALU = mybir.AluOpType
ACTF = mybir.ActivationFunctionType
MAGIC = 8388608.0  # 2^23


def _enc_pm(nimg=IMGS_PER_CORE):
    """Which rows are +-1 coded (ACT Sign), per image (the DVE/ACT A-row
    split alternates by group parity)."""
    g = np.arange(nimg) // GROUP
    a_dve = A_DVE_ROWS + (g & 1)                       # [img]
    rows = np.arange(16)[None, :]
    encA = rows > a_dve[:, None]                       # [img, 16]
    encB = np.zeros((nimg, 16), dtype=bool)
    if B_ACT_ROWS:
        encB[:, 16 - B_ACT_ROWS:] = True
    return encA, encB


# ---------------------------------------------------------------------------
# Pass A: histogram + ci8 cache
def build_hist_nc(nimg=IMGS_PER_CORE, cpi=CPI, group=GROUP):
    assert cpi % 8 == 0
    nc = bacc.Bacc("TRN2", target_bir_lowering=False, debug=False)
    x_d = nc.dram_tensor("x", [PARTS, nimg, cpi], FP32, kind="ExternalInput")
    w_d = nc.dram_tensor("w_raw", [nimg, PARTS, PARTS], FP32, kind="ExternalOutput")
    c8_d = nc.dram_tensor("ci8", [PARTS, nimg, cpi], I16, kind="ExternalOutput")

    with TileContext(nc) as tc:
        with (
            tc.tile_pool(name="const", bufs=1) as cpool,
            tc.tile_pool(name="xin", bufs=3) as xpool,
            tc.tile_pool(name="prep", bufs=3) as ppool,
            tc.tile_pool(name="thermo", bufs=2) as tpool,
            tc.tile_pool(name="psum", bufs=6, space="PSUM") as qpool,
        ):
            # per-partition biases for ACT Sign rows: sign(v - cut + 0.5)
            nACT = 15 - A_DVE_ROWS
            act_bias = cpool.tile([PARTS, max(nACT + 1, 1)], FP32, tag="abias")
            ab_idx = {}
            j = 0
            for tau in range(A_DVE_ROWS, 16):
                nc.vector.memset(act_bias[:, j:j + 1], 0.5 - 16 * tau)
                ab_idx[("A", tau)] = j
                j += 1

            n_groups = math.ceil(nimg / group)

            def emit_load(g):
                g0 = g * group
                g1 = min(g0 + group, nimg)
                gw = (g1 - g0) * cpi
                x_t = xpool.tile([PARTS, group * cpi], FP32, tag="x")
                nc.sync.dma_start(
                    out=x_t[:, :gw],
                    in_=bass.AP(x_d, g0 * cpi, [[nimg * cpi, PARTS], [1, gw]]),
                )
                return x_t

            def emit_prep(g, x_t):
                """DVE floor prep.  CAUTION: DVE op0/op1 chains are FUSED
                (no intermediate fp32 rounding — probe-verified), so the
                255* multiply must be a LONE op to round fl(255x) before
                the magic add; the add-add chain itself is safe fused
                (s-0.5 exact, convert rounds nearest) or unfused (classic
                2^23 magic)."""
                g0 = g * group
                g1 = min(g0 + group, nimg)
                gw = (g1 - g0) * cpi
                # +8 pad: A-plane pack count must be EVEN for DVE 4x mode
                # (odd-count [1,8]-run writes drop to 2x — probe-measured)
                ci = ppool.tile([PARTS, group * cpi + 8], I16, tag="ci")
                nc.vector.tensor_scalar(
                    out=x_t[:, :gw], in0=x_t[:, :gw],
                    scalar1=255.0, scalar2=None, op0=ALU.mult,
                )
                nc.vector.tensor_scalar(
                    out=ci[:, :gw], in0=x_t[:, :gw],
                    scalar1=MAGIC - 0.5, scalar2=-MAGIC,
                    op0=ALU.add, op1=ALU.add,
                )
                lo = ppool.tile([PARTS, group * cpi + 8], I16, tag="lo")
                nc.vector.tensor_scalar(
                    out=lo[:, :gw], in0=ci[:, :gw],
                    scalar1=15, scalar2=None, op0=ALU.bitwise_and,
                )
                return ci, lo

            x_cur = emit_load(0)
            x_nxt = emit_load(1) if n_groups > 1 else None
            # ones rows are CONSTANT: write them once per physical tile
            # buffer (pool rotation: 2 requests = both buffers), then never
            # again — keeps the slow, variable GPSIMD memsets off the
            # steady-state critical path (they were serializing each
            # group's A/B writers via whole-tile WAW tracking).
            for _ in range(2):
                A_pre = tpool.tile([PARTS, group * cpi // 8 + 1, 16, 8], BF16, tag="A")
                B_pre = tpool.tile([PARTS, 16, group * cpi], BF16, tag="B")
                nc.gpsimd.memset(A_pre[:, :, 0, :], 1.0)
                nc.gpsimd.memset(B_pre[:, 0, :], 1.0)

            cur = emit_prep(0, x_cur)
            pend = []
            for g in range(n_groups):
                g0 = g * group
                g1 = min(g0 + group, nimg)
                gw = (g1 - g0) * cpi
                gw8 = gw // 8
                gw8p = gw8 + (gw8 & 1)   # even pack count for DVE 4x
                ci, lo = cur

                if g + 2 < n_groups:
                    x_nxt2 = emit_load(g + 2)
                else:
                    x_nxt2 = None

                A_t = tpool.tile([PARTS, group * cpi // 8 + 1, 16, 8], BF16, tag="A")
                B_t = tpool.tile([PARTS, 16, group * cpi], BF16, tag="B")
                # alternate the DVE/ACT A-row split by group parity to
                # balance the two engines (7.5 rows each on average)
                a_dve = A_DVE_ROWS + (g & 1)

                ci_v = ci[:, :gw8p * 8].rearrange("p (a b) -> p a b", b=8)
                # ACT: Sign rows of the A plane + the ci8 u8 conversion.
                # Emitted before DVE's blocks so ACT is done with the A tile
                # by the time DVE's A rows issue (single-writer windows).
                for tau in range(a_dve + 1, 16):
                    j = ab_idx[("A", tau)]
                    nc.scalar.activation(
                        out=A_t[:, :gw8p, tau, :], in_=ci_v,
                        func=ACTF.Sign,
                        bias=act_bias[:, j:j + 1],
                        scale=1.0,
                    )
                nc.sync.dma_start(
                    out=bass.AP(c8_d, g0 * cpi, [[nimg * cpi, PARTS], [1, gw]]),
                    in_=ci[:, :gw],
                )

                # DVE: B plane rows
                for sg in range(1, 16):
                    nc.vector.tensor_scalar(
                        out=B_t[:, sg, :gw], in0=lo[:, :gw],
                        scalar1=sg, scalar2=None, op0=ALU.is_ge,
                    )

                # filler between DVE's B and A blocks: next group's prep
                if g + 1 < n_groups:
                    cur = emit_prep(g + 1, x_nxt)
                x_nxt = x_nxt2

                # DVE: A plane rows (ACT has finished its A rows by now)
                for tau in range(1, a_dve + 1):
                    nc.vector.tensor_scalar(
                        out=A_t[:, :gw8p, tau, :], in0=ci_v,
                        scalar1=16 * tau, scalar2=None, op0=ALU.is_ge,
                    )

                # PE: per image, 49 packed [128,128] matmuls accumulate in
                # PSUM.  The psum->SBUF copies are DEFERRED one group (psum
                # bufs=8 holds 2 groups) so ACT's in-order queue never
                # head-of-line blocks its next feature rows on an MM wait.
                packs_per_img = cpi // 8
                for i in range(g0, g1):
                    il = i - g0
                    psum_t = qpool.tile([PARTS, PARTS], FP32, tag="ps")
                    for k in range(packs_per_img):
                        p = il * packs_per_img + k
                        nc.tensor.matmul(
                            psum_t[:],
                            A_t[:, p, :, :].rearrange("p a b -> p (a b)"),
                            B_t[:, :, 8 * p:8 * p + 8],
                            start=(k == 0),
                            stop=(k == packs_per_img - 1),
                        )
                    pend.append((i, psum_t))
                while len(pend) > group:
                    i, psum_t = pend.pop(0)
                    w_sb = ppool.tile([PARTS, PARTS], FP32, tag="wsb")
                    nc.scalar.copy(w_sb[:], psum_t[:])
                    nc.sync.dma_start(out=w_d.ap()[i], in_=w_sb[:])
            for i, psum_t in pend:
                w_sb = ppool.tile([PARTS, PARTS], FP32, tag="wsb")
                nc.scalar.copy(w_sb[:], psum_t[:])
                nc.sync.dma_start(out=w_d.ap()[i], in_=w_sb[:])
    nc.finalize()
    return nc


# ---------------------------------------------------------------------------
# Pass B: mask from cached ci8
def build_mask_nc(nimg=IMGS_PER_CORE, cpi=CPI, mgroup=MGROUP):
    nc = bacc.Bacc("TRN2", target_bir_lowering=False, debug=False)
    c8_d = nc.dram_tensor("ci8", [PARTS, nimg, cpi], I16, kind="ExternalInput")
    t_d = nc.dram_tensor("thp1", [PARTS, nimg], FP32, kind="ExternalInput")
    m_d = nc.dram_tensor("mask", [PARTS, nimg, cpi], U8, kind="ExternalOutput")

    with TileContext(nc) as tc:
        with (
            tc.tile_pool(name="cst", bufs=1) as cpool,
            tc.tile_pool(name="cin", bufs=4) as xpool,
            tc.tile_pool(name="mo", bufs=4) as mpool,
        ):
            th_all = cpool.tile([PARTS, nimg], FP32, tag="t")
            nc.sync.dma_start(out=th_all[:], in_=t_d.ap())
            for g0 in range(0, nimg, mgroup):
                g1 = min(g0 + mgroup, nimg)
                gl = g1 - g0
                c_t = xpool.tile([PARTS, mgroup * cpi], I16, tag="c")
                m_t = mpool.tile([PARTS, mgroup * cpi], U8, tag="m")
                nc.sync.dma_start(
                    out=c_t[:, :gl * cpi],
                    in_=bass.AP(
                        c8_d, g0 * cpi,
                        [[nimg * cpi, PARTS], [1, gl * cpi]],
                    ),
                )
                for i in range(g0, g1):
                    il = i - g0
                    nc.vector.tensor_scalar(
                        out=m_t[:, il * cpi:(il + 1) * cpi],
                        in0=c_t[:, il * cpi:(il + 1) * cpi],
                        scalar1=th_all[:, i:i + 1],
                        scalar2=None, op0=ALU.is_ge,
                    )
                nc.sync.dma_start(
                    out=bass.AP(
                        m_d, g0 * cpi,
                        [[nimg * cpi, PARTS], [1, gl * cpi]],
                    ),
                    in_=m_t[:, :gl * cpi],
                )
    nc.finalize()
    return nc


# ---------------------------------------------------------------------------
# Host: decode W, exact-float32 Otsu
def decode_hist(w_raw, nimg=IMGS_PER_CORE, npix=PIX):
    """w_raw [nimg, 128, 128] fp32 -> hist [nimg, 256] int64 (exact).

    Psum row 8*tau+c', col 8*sigma+c'': sum the c'==c'' diagonal blocks."""
    encA, encB = _enc_pm(nimg)               # [img, 16] each
    P128 = np.round(np.asarray(w_raw, np.float64)).astype(np.int64)
    P128 = P128.reshape(nimg, 16, 8, 16, 8)  # [img, tau, c', sigma, c'']
    R = np.einsum("itcsc->its", P128)        # [img, tau, sigma]
    P = npix
    sumB = np.where(encB, (R[:, 0, :] + P) // 2, R[:, 0, :])
    sumA = np.where(encA, (R[:, :, 0] + P) // 2, R[:, :, 0])
    eA = encA[:, :, None]
    eB = encB[:, None, :]
    sA = sumA[:, :, None]
    sB = sumB[:, None, :]
    W = np.where(
        ~eA & ~eB, R,
        np.where(
            eA & ~eB, (R + sB) // 2,
            np.where(~eA & eB, (R + sA) // 2, (R + 2 * sA + 2 * sB - P) // 4),
        ),
    )
    chk = np.where(
        ~eA & ~eB, 0,
        np.where(eA & ~eB, (R + sB) % 2,
                 np.where(~eA & eB, (R + sA) % 2, (R + 2 * sA + 2 * sB - P) % 4)),
    )
    assert not chk.any(), "non-integer decode: device histogram corrupted"
    Wp = np.zeros((nimg, 17, 17), np.int64)
    Wp[:, :16, :16] = W
    hist = (Wp[:, :16, :16] - Wp[:, 1:, :16] - Wp[:, :16, 1:] + Wp[:, 1:, 1:])
    hist = hist.reshape(nimg, 256)
    assert (hist >= 0).all() and (hist.sum(1) == P).all(), "bad histogram"
    return hist


def otsu_f32(hist):
    """Mirror the jax float32 reference exactly. hist [n,256] int64 -> th, bad."""
    f = hist.astype(np.float32)
    centers = np.arange(NBINS, dtype=np.float32)
    w1 = np.cumsum(f, axis=1, dtype=np.float32)
    total = w1[:, -1:]
    s1 = np.cumsum(f * centers, axis=1, dtype=np.float32)
    stot = s1[:, -1:]
    w2 = total - w1
    with np.errstate(divide="ignore", invalid="ignore"):
        m1 = s1 / w1
        m2 = (stot - s1) / w2
        d = m1 - m2
        var12 = (w1 * w2) * (d * d)
    nz = hist > 0
    t = np.arange(NBINS)
    vmin = np.argmax(nz, axis=1)
    vmax = NBINS - 1 - np.argmax(nz[:, ::-1], axis=1)
    valid = (t[None, :] >= vmin[:, None]) & (t[None, :] < vmax[:, None])
    var12 = np.where(valid, var12, np.float32(-1.0))
    th = np.argmax(var12, axis=1)
    th = np.where(th == 0, 1, th)
    th = np.where(th == 255, 254, th)
    bad = vmin == vmax
    return th, bad


# ---------------------------------------------------------------------------
_NC_CACHE = {}


def _get_ncs():
    if "hist" not in _NC_CACHE:
        _NC_CACHE["hist"] = build_hist_nc()
        _NC_CACHE["mask"] = build_mask_nc()
    return _NC_CACHE["hist"], _NC_CACHE["mask"]


def kernel(x: np.ndarray, _profile: dict | None = None) -> np.ndarray:
    x = np.ascontiguousarray(np.asarray(x, dtype=np.float32))
    assert x.shape == (B, N, H, W_IMG)
    nc_hist, nc_mask = _get_ncs()

    bpc = B // N_CORES
    shards = [
        np.ascontiguousarray(
            x[k * bpc:(k + 1) * bpc]
            .reshape(IMGS_PER_CORE, PARTS, CPI)
            .transpose(1, 0, 2)
        )
        for k in range(N_CORES)
    ]
    core_ids = list(range(N_CORES))

    kwargs_a = dict(_profile.get("a", {})) if _profile else {}
    res_a = run_bass_kernel_spmd(
        nc_hist, [{"x": s} for s in shards], core_ids=core_ids, **kwargs_a
    )
    if _profile is not None:
        _profile["res_a"] = res_a

    thp1s = []
    for k in range(N_CORES):
        hist = decode_hist(res_a.results[k]["w_raw"])
        th, bad = otsu_f32(hist)
        thp1 = np.where(bad, np.float32(256.0), (th + 1).astype(np.float32))
        thp1s.append(
            np.ascontiguousarray(
                np.broadcast_to(thp1[None, :], (PARTS, IMGS_PER_CORE))
            ).astype(np.float32)
        )

    kwargs_b = dict(_profile.get("b", {})) if _profile else {}
    res_b = run_bass_kernel_spmd(
        nc_mask,
        [{"ci8": np.asarray(res_a.results[k]["ci8"]), "thp1": thp1s[k]}
         for k in range(N_CORES)],
        core_ids=core_ids,
        **kwargs_b,
    )
    if _profile is not None:
        _profile["res_b"] = res_b

    out = np.empty((B, N, H, W_IMG), np.int32)
    for k in range(N_CORES):
        m = np.asarray(res_b.results[k]["mask"])  # [128, 64, 392] u8
        out[k * bpc:(k + 1) * bpc] = (
            m.astype(np.int32).transpose(1, 0, 2).reshape(bpc, N, H, W_IMG)
        )
    return out
